# revision 2
# baseline (speedup 1.0000x reference)
import sys

sys.path.insert(0, "/opt/trn_rl_repo")

import copy

import numpy as np
import ml_dtypes

import concourse.bass as bass
import concourse.mybir as mybir
from concourse.tile import TileContext
from concourse.bass_utils import run_bass_kernel_spmd


def _split_multiwait_drains(nc):
    """This walrus build only encodes one sem-wait per instruction; hoist
    extra waits onto preceding same-engine NoOps (engines execute their
    instructions in block order, so the waits remain equivalent)."""
    import bass_rust

    uid = [0]
    for fn in nc.m.functions:
        for blk in fn.blocks:
            out, changed = [], False
            for inst in blk.instructions:
                si = getattr(inst, "sync_info", None)
                if si is not None and si.on_wait and len(si.on_wait) > 1:
                    waits = list(si.on_wait)
                    for w in waits[:-1]:
                        n = bass_rust.InstNoOp(name=f"syncw_{uid[0]}", ins=[], outs=[])
                        uid[0] += 1
                        n.engine = inst.engine
                        n.sync_info = bass_rust.SyncInfo(on_wait=[w], on_update=[])
                        out.append(n)
                    si.on_wait = [waits[-1]]
                    changed = True
                out.append(inst)
            if changed:
                blk.instructions = out

import os

B, C, H, W = 4, 128, 128, 128
N_HSEQ = int(os.environ.get("AXIAL_NH", "64"))
N_WSEQ = int(os.environ.get("AXIAL_NW", "128"))
USE_LN = os.environ.get("AXIAL_LN", "1") == "1"
USE_TP = os.environ.get("AXIAL_TP", "1") == "1"
STAGE = int(os.environ.get("AXIAL_STAGE", "5"))
HEADS, DH = 8, 16
WL = 64  # per-core w-slice (2 cores per batch image)
N_CORES = 8

FP32 = mybir.dt.float32
BF16 = mybir.dt.bfloat16
BF16_NP = ml_dtypes.bfloat16

AXES = ("h", "w")


def _build_nc():
    nc = bass.Bass()

    x = nc.declare_dram_parameter("x", [C, H * W], FP32, isOutput=False)
    phw = nc.declare_dram_parameter("phw", [C, H * W], BF16, isOutput=False)
    sel = nc.declare_dram_parameter("sel", [C, C], BF16, isOutput=False)
    bosum = nc.declare_dram_parameter("bosum", [C, 1], FP32, isOutput=False)
    wts = {}
    for ax in AXES:
        for wn in ("wka", "wkb", "woa", "wob") + tuple(f"wqm{h}" for h in range(8)):
            wts[f"{wn}_{ax}"] = nc.declare_dram_parameter(
                f"{wn}_{ax}", [C, C], BF16, isOutput=False
            )
        wts[f"wv_{ax}"] = nc.declare_dram_parameter(
            f"wv_{ax}", [C, 2 * C], BF16, isOutput=False
        )
    out = nc.declare_dram_parameter("out", [C, H * WL], FP32, isOutput=True)

    EXP = mybir.ActivationFunctionType.Exp
    LOG = mybir.ActivationFunctionType.Ln
    ADD = mybir.AluOpType.add
    MULT = mybir.AluOpType.mult

    with TileContext(nc) as tc:
        with (
            tc.tile_pool(name="big", bufs=1) as big,
            tc.tile_pool(name="wpool", bufs=1) as wpool,
            tc.tile_pool(name="qksb", bufs=3) as qksb_pool,
            tc.tile_pool(name="etsb", bufs=3) as etsb_pool,
            tc.tile_pool(name="misc", bufs=3) as misc_pool,
            tc.tile_pool(name="qkps", bufs=1, space="PSUM") as qkps_pool,
            tc.tile_pool(name="kps", bufs=1, space="PSUM") as kps_pool,
            tc.tile_pool(name="vps", bufs=1, space="PSUM") as vps_pool,
            tc.tile_pool(name="scps", bufs=1, space="PSUM") as scps_pool,
            tc.tile_pool(name="ups", bufs=1, space="PSUM") as ups_pool,
            tc.tile_pool(name="rbps", bufs=1, space="PSUM") as rbps_pool,
            tc.tile_pool(name="yps", bufs=1, space="PSUM") as yps_pool,
        ):
            # ---- stage weights ----
            wsb = {}
            for k, t in wts.items():
                wt = wpool.tile(list(t.shape), t.dtype, tag=k)
                nc.gpsimd.dma_start(out=wt[:], in_=t[:])
                wsb[k] = wt
            sel_sb = wpool.tile([C, C], BF16, tag="sel")
            nc.gpsimd.dma_start(out=sel_sb[:], in_=sel[:])
            bos_sb = wpool.tile([C, 1], FP32, tag="bos")
            nc.gpsimd.dma_start(out=bos_sb[:], in_=bosum[:])

            # ---- resident slabs ----
            x_sb = big.tile([C, H * W], FP32, tag="x")
            xp_sb = big.tile([C, H * W], BF16, tag="xp")
            th_sb = big.tile([C, H * WL], BF16, tag="th")  # [c, h, wl]
            tw_sb = big.tile([C, H * WL], BF16, tag="tw")  # [c, h, wl]
            out_sb = big.tile([C, H * WL], FP32, tag="osb")
            nc.vector.memset(th_sb[:], 0.0)
            nc.vector.memset(tw_sb[:], 0.0)

            # xp = x + phw (chunked for overlap)
            NCH = 8
            CH = (H * W) // NCH
            for k in range(NCH):
                cs = slice(k * CH, (k + 1) * CH)
                nc.gpsimd.dma_start(out=x_sb[:, cs], in_=x[:, cs])
                nc.gpsimd.dma_start(out=xp_sb[:, cs], in_=phw[:, cs])
                nc.vector.tensor_tensor(
                    out=xp_sb[:, cs], in0=xp_sb[:, cs], in1=x_sb[:, cs], op=ADD
                )

            # v tiles with persistent ones columns
            v_tiles = []
            for j in range(3):
                vt = wpool.tile([C, 2 * C], BF16, tag=f"vsb{j}")
                vt3 = vt[:].rearrange("p (h c) -> p h c", c=32)
                # ones in cols 32h+16 .. 32h+31 (so u junk rows = finite sums)
                nc.vector.memset(vt3[:, :, 16:], 1.0)
                v_tiles.append(vt)

            def attn_seq(ax, rhs_q, rhs_kv, nq, seq_idx):
                """one attention sequence; writes y (=WoT @ normalized o) into y_ps cols."""
                m = rhs_kv.shape[-1]
                # --- projections: per-head masked q (8) + ka kb
                q_ps = qkps_pool.tile([C, 8 * nq], FP32, tag="qps")
                for h in range(8):
                    nc.tensor.matmul(q_ps[:, h * nq : (h + 1) * nq], wsb[f"wqm{h}_{ax}"][:], rhs_q)
                k_ps = kps_pool.tile([C, 2 * m], FP32, tag="kps")
                nc.tensor.matmul(k_ps[:, 0:m], wsb[f"wka_{ax}"][:], rhs_kv)
                nc.tensor.matmul(k_ps[:, m:], wsb[f"wkb_{ax}"][:], rhs_kv)
                q_sb = qksb_pool.tile([C, 8 * nq], BF16, tag="qsb")
                nc.vector.tensor_copy(q_sb[:], q_ps[:])
                k_sb = qksb_pool.tile([C, 2 * m], BF16, tag="ksb")
                nc.vector.tensor_copy(k_sb[:], k_ps[:])
                if STAGE < 1:
                    return None

                # --- v projection (+ ones cols preset in persistent tiles)
                v_ps = vps_pool.tile([C, 2 * C], FP32, tag="v")
                nc.tensor.matmul(v_ps[:], rhs_kv, wsb[f"wv_{ax}"][:])
                v_sb = v_tiles[seq_idx % 3]
                vap_src = v_ps[:].rearrange("p (h c) -> p h c", c=32)[:, :, :16]
                vap_dst = v_sb[:].rearrange("p (h c) -> p h c", c=32)[:, :, :16]
                nc.scalar.copy(vap_dst, vap_src)
                if STAGE < 2:
                    return None

                # --- scores + exp, heads 0-3 (A) then 4-7 (B)
                et_sb = etsb_pool.tile([C, 8 * nq], BF16, tag="et")
                for half in range(2):
                    sc_ps = scps_pool.tile([C, 4 * nq], FP32, tag="sc")
                    for g in range(4):
                        h = half * 4 + g
                        nc.tensor.matmul(
                            sc_ps[:, g * nq : (g + 1) * nq],
                            k_sb[:, half * m : (half + 1) * m],
                            q_sb[:, h * nq : (h + 1) * nq],
                        )
                    nc.scalar.activation(
                        et_sb[:, half * 4 * nq : (half + 1) * 4 * nq], sc_ps[:], EXP
                    )

                if STAGE < 3:
                    return None
                u_col_off = 0
                u_ps = ups_pool.tile([C, 2 * nq], FP32, tag="u")
                # --- o-matmuls (v stationary, col-tiled) -> u [c-grouped, 2*nq]
                for h in range(8):
                    g, half = h % 4, h // 4
                    if USE_TP:
                        nc.tensor.matmul(
                            u_ps[32 * g : 32 * g + 32, u_col_off + half * nq : u_col_off + (half + 1) * nq],
                            v_sb[:, 32 * h : 32 * h + 32],
                            et_sb[:, h * nq : (h + 1) * nq],
                            tile_position=(0, 32 * g),
                        )
                    else:
                        nc.tensor.matmul(
                            u_ps[32 * g : 32 * g + 32, u_col_off + half * nq : u_col_off + (half + 1) * nq],
                            v_sb[:, 32 * h : 32 * h + 32],
                            et_sb[:, h * nq : (h + 1) * nq],
                        )

                if STAGE < 4:
                    return None
                # --- normalize: broadcast sums (rows 32g+16) via SEL matmul, recip, mult
                uc = slice(u_col_off, u_col_off + 2 * nq)
                u_sb = misc_pool.tile([C, 2 * nq], BF16, tag="usb")
                nc.vector.tensor_copy(u_sb[:], u_ps[:, uc])
                rb_ps = rbps_pool.tile([C, 2 * nq], FP32, tag="rb")
                nc.tensor.matmul(rb_ps[:], sel_sb[:], u_sb[:])
                # 1/s = exp(-ln(s)); Ln+Exp share one ACT table set with the
                # softmax exp, and s is always in [~70, ~220] (sums of exps)
                ls_sb = misc_pool.tile([C, 2 * nq], FP32, tag="ls")
                rn_sb = misc_pool.tile([C, 2 * nq], FP32, tag="rn")
                if USE_LN:
                    nc.scalar.activation(ls_sb[:], rb_ps[:], LOG)
                    nc.scalar.activation(rn_sb[:], ls_sb[:], EXP, scale=-1.0)
                else:
                    nc.scalar.copy(rn_sb[:], rb_ps[:])  # probe: skip recip
                on_sb = misc_pool.tile([C, 2 * nq], BF16, tag="on")
                nc.vector.tensor_tensor(out=on_sb[:], in0=u_sb[:], in1=rn_sb[:], op=MULT)

                if STAGE < 5:
                    return None
                y_ps = yps_pool.tile([C, nq], FP32, tag="y")
                y_col_off = 0
                # --- output projection (grouped Wo, junk rows are zero)
                yc = slice(y_col_off, y_col_off + nq)
                nc.tensor.matmul(y_ps[:, yc], wsb[f"woa_{ax}"][:], on_sb[:, 0:nq], start=True, stop=False)
                nc.tensor.matmul(y_ps[:, yc], wsb[f"wob_{ax}"][:], on_sb[:, nq : 2 * nq], start=False, stop=True)
                return y_ps

            # ================= H-axis: 64 seqs (one per local w) =================
            xp3 = xp_sb[:].rearrange("p (h w) -> p h w", w=W)
            th3 = th_sb[:].rearrange("p (h w) -> p h w", w=WL)
            for wl in range(N_HSEQ):
                rhs = xp3[:, :, wl]  # [C, H] strided
                y_ps = attn_seq("h", rhs, rhs, H, wl)
                if STAGE >= 5:
                    nc.scalar.copy(th3[:, :, wl], y_ps[:])

            # ================= W-axis: 128 seqs (one per h row) =================
            for h in range(N_WSEQ):
                rhs_kv = xp3[:, h, :]  # [C, W] contiguous
                rhs_q = xp3[:, h, 0:WL]
                y_ps = attn_seq("w", rhs_q, rhs_kv, WL, h)
                if STAGE >= 5:
                    nc.scalar.copy(tw_sb[:, h * WL : (h + 1) * WL], y_ps[:])

            # ================= final: out = th + tw + x + bias =================
            x3 = x_sb[:].rearrange("p (h w) -> p h w", w=W)
            NFC = 8
            FH = H // NFC  # h-rows per chunk
            for k in range(NFC):
                cs = slice(k * FH * WL, (k + 1) * FH * WL)
                t1 = misc_pool.tile([C, FH * WL], BF16, tag="f1")
                nc.vector.tensor_tensor(out=t1[:], in0=th_sb[:, cs], in1=tw_sb[:, cs], op=ADD)
                nc.vector.tensor_scalar_add(out=t1[:], in0=t1[:], scalar1=bos_sb[:])
                nc.vector.tensor_tensor(
                    out=out_sb[:, cs].rearrange("p (h w) -> p h w", w=WL),
                    in0=t1[:].rearrange("p (h w) -> p h w", w=WL),
                    in1=x3[:, k * FH : (k + 1) * FH, 0:WL],
                    op=ADD,
                )
                nc.gpsimd.dma_start(out=out[:, cs], in_=out_sb[:, cs])

    _split_multiwait_drains(nc)
    return nc


_NC_CACHE = None


def _get_nc():
    global _NC_CACHE
    if _NC_CACHE is None:
        _NC_CACHE = _build_nc()
    return _NC_CACHE


def _host_prep(x, pos_h, pos_w, weights):
    """build per-core input maps"""
    scale = DH ** -0.5
    phw = (pos_h + pos_w)[0]  # [C, H, W]

    def grouped_cols(Wm, heads_sel):
        """[C, C] -> zero-padded A/B projection weight: head g -> cols 32g..32g+15"""
        out = np.zeros((C, C), np.float32)
        for g, h in enumerate(heads_sel):
            out[:, 32 * g : 32 * g + 16] = Wm[:, 16 * h : 16 * h + 16]
        return out

    def grouped_rows(Wm, heads_sel):
        out = np.zeros((C, C), np.float32)
        for g, h in enumerate(heads_sel):
            out[32 * g : 32 * g + 16, :] = Wm[16 * h : 16 * h + 16, :]
        return out

    def v_aug(Wm):
        out = np.zeros((C, 2 * C), np.float32)
        for h in range(HEADS):
            out[:, 32 * h : 32 * h + 16] = Wm[:, 16 * h : 16 * h + 16]
        return out

    sel = np.zeros((C, C), np.float32)
    for q in range(C):
        sel[32 * (q // 32) + 16, q] = 1.0

    base = {"sel": sel.astype(BF16_NP)}
    for ax in AXES:
        Wq, Wk, Wv, Wo = weights[ax]
        Wqs = Wq * scale
        for h in range(HEADS):
            wm = np.zeros((C, C), np.float32)
            g = h % 4
            wm[:, 32 * g : 32 * g + 16] = Wqs[:, 16 * h : 16 * h + 16]
            base[f"wqm{h}_{ax}"] = wm.astype(BF16_NP)
        base[f"wka_{ax}"] = grouped_cols(Wk, [0, 1, 2, 3]).astype(BF16_NP)
        base[f"wkb_{ax}"] = grouped_cols(Wk, [4, 5, 6, 7]).astype(BF16_NP)
        base[f"wv_{ax}"] = v_aug(Wv).astype(BF16_NP)
        base[f"woa_{ax}"] = grouped_rows(Wo, [0, 1, 2, 3]).astype(BF16_NP)
        base[f"wob_{ax}"] = grouped_rows(Wo, [4, 5, 6, 7]).astype(BF16_NP)

    in_maps = []
    for core in range(N_CORES):
        b, s = core // 2, core % 2
        xb = x[b]
        pb = phw
        if s == 1:
            xb = np.concatenate([xb[:, :, WL:], xb[:, :, :WL]], axis=2)
            pb = np.concatenate([pb[:, :, WL:], pb[:, :, :WL]], axis=2)
        m = dict(base)
        m["x"] = np.ascontiguousarray(xb.reshape(C, H * W), np.float32)
        m["phw"] = np.ascontiguousarray(pb.reshape(C, H * W)).astype(BF16_NP)
        in_maps.append(m)
    return in_maps


LAST_RESULT = None


def kernel(**inputs):
    x = np.asarray(inputs["x"], np.float32)
    pos_h = np.asarray(inputs["pos_h"], np.float32)
    pos_w = np.asarray(inputs["pos_w"], np.float32)
    weights = {
        "h": tuple(np.asarray(inputs[f"W{t}_h"], np.float32) for t in "qkvo"),
        "w": tuple(np.asarray(inputs[f"W{t}_w"], np.float32) for t in "qkvo"),
    }
    bosum = (
        np.asarray(inputs["bo_h"], np.float32) + np.asarray(inputs["bo_w"], np.float32)
    ).reshape(C, 1)

    in_maps = _host_prep(x, pos_h, pos_w, weights)
    for m in in_maps:
        m["bosum"] = bosum

    nc = _get_nc()
    kw = {}
    if os.environ.get("AXIAL_TRACE") == "1":
        kw["trace"] = True
        td = os.environ.get("AXIAL_TMPDIR")
        if td:
            kw["tmpdir"] = td
    res = run_bass_kernel_spmd(nc, in_maps, list(range(N_CORES)), **kw)
    global LAST_RESULT
    LAST_RESULT = res

    out = np.empty((B, C, H, W), np.float32)
    for core in range(N_CORES):
        b, s = core // 2, core % 2
        o = res.results[core]["out"].reshape(C, H, WL)
        out[b, :, :, s * WL : (s + 1) * WL] = o
    return out


if __name__ == "__main__":
    import reference

    inputs = {k: np.asarray(v) for k, v in reference.setup_inputs().items()}
    got = kernel(**inputs)
    import jax

    with jax.default_device(jax.devices("cpu")[0]):
        exp = np.asarray(reference.reference(**reference.setup_inputs()))
    err = np.abs(got - exp).max() / np.abs(exp).max()
    print("rel err:", err)



# revision 9
# speedup vs baseline: 1.2213x; 1.2213x over previous
import sys

sys.path.insert(0, "/opt/trn_rl_repo")

import os

import numpy as np
import ml_dtypes

import concourse.bass as bass
import concourse.mybir as mybir
from concourse.tile import TileContext
from concourse.bass_utils import run_bass_kernel_spmd


def _split_multiwait_drains(nc):
    """This walrus build only encodes one sem-wait per instruction; hoist
    extra waits onto preceding same-engine NoOps (engines execute their
    instructions in block order, so the waits remain equivalent)."""
    import bass_rust

    uid = [0]
    for fn in nc.m.functions:
        for blk in fn.blocks:
            out, changed = [], False
            for inst in blk.instructions:
                si = getattr(inst, "sync_info", None)
                if si is not None and si.on_wait and len(si.on_wait) > 1:
                    waits = list(si.on_wait)
                    for w in waits[:-1]:
                        n = bass_rust.InstNoOp(name=f"syncw_{uid[0]}", ins=[], outs=[])
                        uid[0] += 1
                        n.engine = inst.engine
                        n.sync_info = bass_rust.SyncInfo(on_wait=[w], on_update=[])
                        out.append(n)
                    si.on_wait = [waits[-1]]
                    changed = True
                out.append(inst)
            if changed:
                blk.instructions = out


B, C, H, W = 4, 128, 128, 128
HEADS, DH = 8, 16
WL = 64  # per-core w-slice (2 cores per batch image)
N_CORES = 8
NB = 2  # seqs per normalization/o-proj batch

FP32 = mybir.dt.float32
BF16 = mybir.dt.bfloat16
BF16_NP = ml_dtypes.bfloat16

EXP = mybir.ActivationFunctionType.Exp
LOG = mybir.ActivationFunctionType.Ln
ADD = mybir.AluOpType.add
MULT = mybir.AluOpType.mult

AXES = ("h", "w")


def _build_nc():
    nc = bass.Bass()

    xp = nc.declare_dram_parameter("xp", [C, H * W], BF16, isOutput=False)
    xres = nc.declare_dram_parameter("xres", [C, H * WL], BF16, isOutput=False)
    wts = {}
    for ax in AXES:
        for wn in ("woa", "wob", "wv") + tuple(f"gz{h}" for h in range(8)):
            wts[f"{wn}_{ax}"] = nc.declare_dram_parameter(
                f"{wn}_{ax}", [C, C], BF16, isOutput=False
            )
    out = nc.declare_dram_parameter("out", [C, H * WL], FP32, isOutput=True)

    with TileContext(nc) as tc:
        with (
            tc.tile_pool(name="big", bufs=1) as big,
            tc.tile_pool(name="wpool", bufs=1) as wpool,
            tc.tile_pool(name="qk", bufs=2) as qk_pool,
            tc.tile_pool(name="etsb", bufs=3) as et_pool,
            tc.tile_pool(name="vsb", bufs=1) as v_pool,
            tc.tile_pool(name="onsb", bufs=2) as on_pool,
            tc.tile_pool(name="misc", bufs=2) as misc_pool,
            tc.tile_pool(name="outsb", bufs=2) as out_pool,
            tc.tile_pool(name="sps", bufs=2, space="PSUM") as s_pool,
            tc.tile_pool(name="uvps", bufs=2, space="PSUM") as u_pool,
            tc.tile_pool(name="u2yps", bufs=1, space="PSUM") as u2_pool,
        ):
            # ---- stage weights ----
            wsb = {}
            for k, t in wts.items():
                wt = wpool.tile(list(t.shape), t.dtype, tag=k)
                nc.gpsimd.dma_start(out=wt[:], in_=t[:])
                wsb[k] = wt

            # ---- resident slabs ----
            xp_sb = big.tile([C, H * W], BF16, tag="xp")
            xres_sb = big.tile([C, H * WL], BF16, tag="xres")
            th_sb = big.tile([C, H * WL], BF16, tag="th")  # [c, (h, wl)]
            tw_sb = big.tile([C, H * WL], BF16, tag="tw")  # [c, (h, wl)]

            NCH = 8
            CH = (H * W) // NCH
            for k in range(NCH):
                cs = slice(k * CH, (k + 1) * CH)
                nc.gpsimd.dma_start(out=xp_sb[:, cs], in_=xp[:, cs])
            for k in range(4):
                cs = slice(k * (H * WL) // 4, (k + 1) * (H * WL) // 4)
                nc.gpsimd.dma_start(out=xres_sb[:, cs], in_=xres[:, cs])

            # ones stationary for u2 (sum) matmuls
            ones_sb = wpool.tile([C, 32], BF16, tag="ones")
            nc.vector.memset(ones_sb[:], 1.0)

            # v stationary tiles: [keys, 8 heads x 32]; col 32h+16 is ones
            # (feeds the bias-row trick), cols 32h+17.. stay zero.
            v_tiles = []
            for j in range(3):
                vt = v_pool.tile([C, 2 * C], BF16, tag=f"v{j}")
                nc.vector.memset(vt[:], 0.0)
                vt3 = vt[:].rearrange("p (h c) -> p h c", c=32)
                nc.vector.memset(vt3[:, :, 16:17], 1.0)
                v_tiles.append(vt)

            xp3 = xp_sb[:].rearrange("p (h w) -> p h w", w=W)

            # ================= z-stage: z_h = G_h^T xp per chunk =================
            # zh chunk layout [C, (wl8, head, q=H)]; zw [C, (hr8, head, q=WL)]
            def z_stage_h(wl0, zch):
                zh4 = zch[:].rearrange("p (s h q) -> p s h q", h=8, q=H)
                for h in range(8):
                    ps = s_pool.tile([C, 8 * H], FP32, tag="s")
                    for j in range(2):
                        rhs = xp3[:, :, wl0 + 4 * j : wl0 + 4 * (j + 1)].rearrange(
                            "p h w -> p w h"
                        )
                        nc.tensor.matmul(
                            ps[:, j * 512 : (j + 1) * 512], wsb[f"gz{h}_h"][:], rhs
                        )
                    pss = ps[:, 0:1024].rearrange("p (s q) -> p s q", q=H)
                    if h % 2 == 0:
                        nc.scalar.copy(zh4[:, :, h, :], pss)
                    else:
                        nc.vector.tensor_copy(zh4[:, :, h, :], pss)

            def z_stage_w(h0, zch):
                zw4 = zch[:].rearrange("p (s h q) -> p s h q", h=8, q=WL)
                for h in range(8):
                    ps = s_pool.tile([C, 8 * H], FP32, tag="s")
                    rhs = xp3[:, h0 : h0 + 8, 0:WL]
                    nc.tensor.matmul(ps[:, 0:512], wsb[f"gz{h}_w"][:], rhs)
                    pss = ps[:, 0:512].rearrange("p (s q) -> p s q", q=WL)
                    if h % 2 == 0:
                        nc.scalar.copy(zw4[:, :, h, :], pss)
                    else:
                        nc.vector.tensor_copy(zw4[:, :, h, :], pss)

            def attn_seq(ax, nq, z_seq, xp_key_ap, seq_idx, u2_sl, u2_batch):
                """one attention sequence through attn@v; returns u_ps."""
                # --- scores: S[m, (h, q)] = xp_slice^T @ z_slice
                s_ps = s_pool.tile([C, 8 * H], FP32, tag="s")
                ncols = 8 * nq
                for j in range((ncols + 511) // 512):
                    c0, c1 = j * 512, min((j + 1) * 512, ncols)
                    nc.tensor.matmul(
                        s_ps[:, c0:c1], xp_key_ap, z_seq[:, c0:c1]
                    )
                uv = u_pool.tile([C, 2 * H + C], FP32, tag="u")
                # --- v projection: [keys, 128] (16 dims per head, compact)
                v_ps = uv[:, 2 * H : 2 * H + C]
                nc.tensor.matmul(v_ps, xp_key_ap, wsb[f"wv_{ax}"][:])
                v_sb = v_tiles[seq_idx % 3]
                vsrc = v_ps.rearrange("p (h c) -> p h c", c=16)
                vdst = v_sb[:].rearrange("p (h c) -> p h c", c=32)[:, :, 0:16]
                nc.vector.tensor_copy(vdst, vsrc)

                # --- exp
                et_sb = et_pool.tile([C, 8 * H], BF16, tag="et")
                nc.scalar.activation(et_sb[:, : 8 * nq], s_ps[:, : 8 * nq], EXP)

                # --- attn@v (u) and sums (u2), col-tiled
                u_ps = uv[:, 0 : 2 * H]
                for h in range(8):
                    g, half = h % 4, h // 4
                    oc = slice(half * nq, (half + 1) * nq)
                    nc.tensor.matmul(
                        u_ps[32 * g : 32 * g + 32, oc],
                        v_sb[:, 32 * h : 32 * h + 32],
                        et_sb[:, h * nq : (h + 1) * nq],
                        tile_position=(0, 32 * g),
                    )
                for h in range(8):
                    g, half = h % 4, h // 4
                    oc = slice(
                        u2_sl * 2 * nq + half * nq, u2_sl * 2 * nq + (half + 1) * nq
                    )
                    nc.tensor.matmul(
                        u2_batch[32 * g : 32 * g + 32, oc],
                        ones_sb[:],
                        et_sb[:, h * nq : (h + 1) * nq],
                        tile_position=(0, 32 * g),
                    )
                return u_ps

            def normalize(u_list, u2_batch, on_batch, nq):
                nb = len(u_list)
                w = nb * 2 * nq
                ls_sb = misc_pool.tile([C, NB * 2 * H], FP32, tag="ls")
                rn_sb = misc_pool.tile([C, NB * 2 * H], BF16, tag="rn")
                nc.scalar.activation(ls_sb[:, :w], u2_batch[:, :w], LOG)
                nc.scalar.activation(rn_sb[:, :w], ls_sb[:, :w], EXP, scale=-1.0)
                for i, u_ps in enumerate(u_list):
                    cs = slice(i * 2 * nq, (i + 1) * 2 * nq)
                    nc.vector.tensor_tensor(
                        out=on_batch[:, cs],
                        in0=u_ps[:, 0 : 2 * nq],
                        in1=rn_sb[:, cs],
                        op=MULT,
                    )

            def oproj(ax, on_batch, y_ps, nb, nq, y_dst_fn):
                on3 = on_batch[:].rearrange("p (s c) -> p s c", c=2 * nq)
                nc.tensor.matmul(
                    y_ps[:, : nb * nq],
                    wsb[f"woa_{ax}"][:],
                    on3[:, 0:nb, 0:nq],
                    start=True,
                    stop=False,
                )
                nc.tensor.matmul(
                    y_ps[:, : nb * nq],
                    wsb[f"wob_{ax}"][:],
                    on3[:, 0:nb, nq : 2 * nq],
                    start=False,
                    stop=True,
                )
                y_dst_fn(y_ps, nb)

            # ================= H-axis =================
            th3 = th_sb[:].rearrange("p (h w) -> p h w", w=WL)

            for blk in range(WL // NB):
                if blk % (8 // NB) == 0:
                    zch_h = qk_pool.tile([C, 8 * 8 * H], BF16, tag="zh")
                    z_stage_h(blk * NB, zch_h)
                    zh4 = zch_h[:].rearrange("p (s h q) -> p s (h q)", h=8, q=H)
                u2y = u2_pool.tile([C, NB * 2 * H + NB * H], FP32, tag="u2")
                u2_batch = u2y[:, 0 : NB * 2 * H]
                y_ps = u2y[:, NB * 2 * H :]
                on_batch = on_pool.tile([C, NB * 2 * H], BF16, tag="on")
                u_list = []
                for i in range(NB):
                    wl = blk * NB + i
                    u_ps = attn_seq(
                        "h",
                        nq=H,
                        z_seq=zh4[:, wl % 8, :],
                        xp_key_ap=xp3[:, :, wl],
                        seq_idx=wl,
                        u2_sl=i,
                        u2_batch=u2_batch,
                    )
                    u_list.append(u_ps)
                normalize(u_list, u2_batch, on_batch[:], nq=H)

                def y_to_th(y_ps, nb, blk=blk):
                    # y cols = (seq wl, q=h); th layout (h, wl)
                    ysrc = y_ps[:, : nb * H].rearrange("p (w h) -> p h w", h=H)
                    nc.vector.tensor_copy(th3[:, :, blk * NB : blk * NB + nb], ysrc)

                oproj("h", on_batch, y_ps, NB, H, y_to_th)

            # ================= W-axis =================
            tw3 = tw_sb[:].rearrange("p (h w) -> p h w", w=WL)

            for blk in range(H // NB):
                if blk % (8 // NB) == 0:
                    zch_w = qk_pool.tile([C, 8 * 8 * WL], BF16, tag="zw")
                    z_stage_w(blk * NB, zch_w)
                    zw4 = zch_w[:].rearrange("p (s h q) -> p s (h q)", h=8, q=WL)
                u2y = u2_pool.tile([C, NB * 2 * H + NB * H], FP32, tag="u2")
                u2_batch = u2y[:, 0 : NB * 2 * H]
                y_ps = u2y[:, NB * 2 * H :]
                on_batch = on_pool.tile([C, NB * 2 * H], BF16, tag="on")
                u_list = []
                for i in range(NB):
                    hr = blk * NB + i
                    u_ps = attn_seq(
                        "w",
                        nq=WL,
                        z_seq=zw4[:, hr % 8, :],
                        xp_key_ap=xp3[:, hr, :],
                        seq_idx=hr,
                        u2_sl=i,
                        u2_batch=u2_batch,
                    )
                    u_list.append(u_ps)
                normalize(u_list, u2_batch, on_batch[:], nq=WL)

                def y_to_tw(y_ps, nb, blk=blk):
                    nc.vector.tensor_copy(
                        tw3[:, blk * NB : blk * NB + nb, :],
                        y_ps[:, : nb * WL].rearrange("p (s w) -> p s w", w=WL),
                    )

                oproj("w", on_batch, y_ps, NB, WL, y_to_tw)

            # ================= final: out = th + tw + xres =================
            NFC = 16
            FC = (H * WL) // NFC
            for k in range(NFC):
                cs = slice(k * FC, (k + 1) * FC)
                t1 = misc_pool.tile([C, FC], BF16, tag="f1")
                nc.vector.tensor_tensor(
                    out=t1[:], in0=th_sb[:, cs], in1=tw_sb[:, cs], op=ADD
                )
                o1 = out_pool.tile([C, FC], FP32, tag="o1")
                nc.vector.tensor_tensor(
                    out=o1[:], in0=t1[:], in1=xres_sb[:, cs], op=ADD
                )
                nc.gpsimd.dma_start(out=out[:, cs], in_=o1[:])

    _split_multiwait_drains(nc)
    return nc


_NC_CACHE = None


def _get_nc():
    global _NC_CACHE
    if _NC_CACHE is None:
        _NC_CACHE = _build_nc()
    return _NC_CACHE


def _host_prep(x, pos_h, pos_w, weights, bo_sum):
    scale = DH ** -0.5
    phw = (pos_h + pos_w)[0]  # [C, H, W]

    def grouped_rows(Wm, heads_sel):
        o = np.zeros((C, C), np.float32)
        for g, h in enumerate(heads_sel):
            o[32 * g : 32 * g + 16, :] = Wm[16 * h : 16 * h + 16, :]
        return o

    base = {}
    for ax in AXES:
        Wq, Wk, Wv, Wo = weights[ax]
        for h in range(8):
            G = scale * (Wq[:, 16 * h : 16 * h + 16] @ Wk[:, 16 * h : 16 * h + 16].T)
            base[f"gz{h}_{ax}"] = G.astype(BF16_NP)
        base[f"wv_{ax}"] = Wv.astype(BF16_NP)  # compact: head h at cols 16h..
        woa = grouped_rows(Wo, [0, 1, 2, 3])
        wob = grouped_rows(Wo, [4, 5, 6, 7])
        if ax == "h":
            # bias via the on==1 rows (32g+16): 8 such rows across A+B
            for g in range(4):
                woa[32 * g + 16, :] = bo_sum / 8.0
                wob[32 * g + 16, :] = bo_sum / 8.0
        base[f"woa_{ax}"] = woa.astype(BF16_NP)
        base[f"wob_{ax}"] = wob.astype(BF16_NP)

    xp_full = (x + phw[None]).astype(BF16_NP)  # [B, C, H, W]

    in_maps = []
    for core in range(N_CORES):
        b, s = core // 2, core % 2
        xb = xp_full[b]
        if s == 1:
            xb = np.concatenate([xb[:, :, WL:], xb[:, :, :WL]], axis=2)
            xr = x[b][:, :, WL:]
        else:
            xr = x[b][:, :, :WL]
        m = dict(base)
        m["xp"] = np.ascontiguousarray(xb.reshape(C, H * W))
        m["xres"] = np.ascontiguousarray(xr.reshape(C, H * WL)).astype(BF16_NP)
        in_maps.append(m)
    return in_maps


LAST_RESULT = None


def kernel(**inputs):
    x = np.asarray(inputs["x"], np.float32)
    pos_h = np.asarray(inputs["pos_h"], np.float32)
    pos_w = np.asarray(inputs["pos_w"], np.float32)
    weights = {
        "h": tuple(np.asarray(inputs[f"W{t}_h"], np.float32) for t in "qkvo"),
        "w": tuple(np.asarray(inputs[f"W{t}_w"], np.float32) for t in "qkvo"),
    }
    bo_sum = np.asarray(inputs["bo_h"], np.float32) + np.asarray(
        inputs["bo_w"], np.float32
    )

    in_maps = _host_prep(x, pos_h, pos_w, weights, bo_sum)

    nc = _get_nc()
    kw = {}
    if os.environ.get("AXIAL_TRACE") == "1":
        kw["trace"] = True
        td = os.environ.get("AXIAL_TMPDIR")
        if td:
            kw["tmpdir"] = td
    res = run_bass_kernel_spmd(nc, in_maps, list(range(N_CORES)), **kw)
    global LAST_RESULT
    LAST_RESULT = res

    out = np.empty((B, C, H, W), np.float32)
    for core in range(N_CORES):
        b, s = core // 2, core % 2
        o = res.results[core]["out"].reshape(C, H, WL)
        out[b, :, :, s * WL : (s + 1) * WL] = o
    return out


if __name__ == "__main__":
    import reference

    inputs = {k: np.asarray(v) for k, v in reference.setup_inputs().items()}
    got = kernel(**inputs)
    import jax

    with jax.default_device(jax.devices("cpu")[0]):
        exp = np.asarray(reference.reference(**reference.setup_inputs()))
    err = np.abs(got - exp).max() / np.abs(exp).max()
    print("rel err:", err)


# revision 10
# speedup vs baseline: 1.4728x; 1.2058x over previous
import sys

sys.path.insert(0, "/opt/trn_rl_repo")

import os

import numpy as np
import ml_dtypes

import concourse.bass as bass
import concourse.mybir as mybir
from concourse.tile import TileContext
from concourse.bass_utils import run_bass_kernel_spmd


def _split_multiwait_drains(nc):
    """This walrus build only encodes one sem-wait per instruction; hoist
    extra waits onto preceding same-engine NoOps (engines execute their
    instructions in block order, so the waits remain equivalent)."""
    import bass_rust

    uid = [0]
    for fn in nc.m.functions:
        for blk in fn.blocks:
            out, changed = [], False
            for inst in blk.instructions:
                si = getattr(inst, "sync_info", None)
                if si is not None and si.on_wait and len(si.on_wait) > 1:
                    waits = list(si.on_wait)
                    for w in waits[:-1]:
                        n = bass_rust.InstNoOp(name=f"syncw_{uid[0]}", ins=[], outs=[])
                        uid[0] += 1
                        n.engine = inst.engine
                        n.sync_info = bass_rust.SyncInfo(on_wait=[w], on_update=[])
                        out.append(n)
                    si.on_wait = [waits[-1]]
                    changed = True
                out.append(inst)
            if changed:
                blk.instructions = out


B, C, H, W = 4, 128, 128, 128
HEADS, DH = 8, 16
WL = 64  # per-core w-slice (2 cores per batch image)
N_CORES = 8
NB = 2  # seqs per normalization/o-proj batch

FP32 = mybir.dt.float32
BF16 = mybir.dt.bfloat16
BF16_NP = ml_dtypes.bfloat16

EXP = mybir.ActivationFunctionType.Exp
LOG = mybir.ActivationFunctionType.Ln
ADD = mybir.AluOpType.add
MULT = mybir.AluOpType.mult

AXES = ("h", "w")


def _build_nc():
    nc = bass.Bass()

    xp = nc.declare_dram_parameter("xp", [C, H * W], BF16, isOutput=False)
    xres = nc.declare_dram_parameter("xres", [C, H * WL], BF16, isOutput=False)
    sel = nc.declare_dram_parameter("sel", [C, C], BF16, isOutput=False)
    wts = {}
    for ax in AXES:
        for wn in ("woa", "wob", "wv") + tuple(f"gz{h}" for h in range(8)):
            wts[f"{wn}_{ax}"] = nc.declare_dram_parameter(
                f"{wn}_{ax}", [C, C], BF16, isOutput=False
            )
    out = nc.declare_dram_parameter("out", [C, H * WL], FP32, isOutput=True)

    with TileContext(nc) as tc:
        with (
            tc.tile_pool(name="big", bufs=1) as big,
            tc.tile_pool(name="wpool", bufs=1) as wpool,
            tc.tile_pool(name="qk", bufs=2) as qk_pool,
            tc.tile_pool(name="etsb", bufs=3) as et_pool,
            tc.tile_pool(name="vsb", bufs=1) as v_pool,
            tc.tile_pool(name="onsb", bufs=2) as on_pool,
            tc.tile_pool(name="misc", bufs=2) as misc_pool,
            tc.tile_pool(name="outsb", bufs=2) as out_pool,
            tc.tile_pool(name="sps", bufs=2, space="PSUM") as s_pool,
            tc.tile_pool(name="uvps", bufs=2, space="PSUM") as u_pool,
            tc.tile_pool(name="u2yps", bufs=1, space="PSUM") as u2_pool,
        ):
            # ---- stage weights ----
            wsb = {}
            for k, t in wts.items():
                wt = wpool.tile(list(t.shape), t.dtype, tag=k)
                nc.gpsimd.dma_start(out=wt[:], in_=t[:])
                wsb[k] = wt

            # ---- resident slabs ----
            xp_sb = big.tile([C, H * W], BF16, tag="xp")
            xres_sb = big.tile([C, H * WL], BF16, tag="xres")
            th_sb = big.tile([C, H * WL], BF16, tag="th")  # [c, (h, wl)]
            tw_sb = big.tile([C, H * WL], BF16, tag="tw")  # [c, (h, wl)]

            NCH = 8
            CH = (H * W) // NCH
            for k in range(NCH):
                cs = slice(k * CH, (k + 1) * CH)
                nc.gpsimd.dma_start(out=xp_sb[:, cs], in_=xp[:, cs])
            for k in range(4):
                cs = slice(k * (H * WL) // 4, (k + 1) * (H * WL) // 4)
                nc.gpsimd.dma_start(out=xres_sb[:, cs], in_=xres[:, cs])

            sel_sb = wpool.tile([C, C], BF16, tag="sel")
            nc.gpsimd.dma_start(out=sel_sb[:], in_=sel[:])

            # v stationary tiles: [keys, 8 heads x 32]; col 32h+16 is ones
            # (feeds the bias-row trick), cols 32h+17.. stay zero.
            v_tiles = []
            for j in range(3):
                vt = v_pool.tile([C, 2 * C], BF16, tag=f"v{j}")
                nc.vector.memset(vt[:], 0.0)
                vt3 = vt[:].rearrange("p (h c) -> p h c", c=32)
                nc.vector.memset(vt3[:, :, 16:17], 1.0)
                v_tiles.append(vt)

            xp3 = xp_sb[:].rearrange("p (h w) -> p h w", w=W)

            # ================= z-stage: z_h = G_h^T xp per chunk =================
            # zh chunk layout [C, (wl8, head, q=H)]; zw [C, (hr8, head, q=WL)]
            def z_stage_h(wl0, zch):
                zh4 = zch[:].rearrange("p (s h q) -> p s h q", h=8, q=H)
                for h in range(8):
                    ps = s_pool.tile([C, 8 * H], FP32, tag="s")
                    for j in range(2):
                        rhs = xp3[:, :, wl0 + 4 * j : wl0 + 4 * (j + 1)].rearrange(
                            "p h w -> p w h"
                        )
                        nc.tensor.matmul(
                            ps[:, j * 512 : (j + 1) * 512], wsb[f"gz{h}_h"][:], rhs
                        )
                    pss = ps[:, 0:1024].rearrange("p (s q) -> p s q", q=H)
                    if h % 2 == 0:
                        nc.scalar.copy(zh4[:, :, h, :], pss)
                    else:
                        nc.vector.tensor_copy(zh4[:, :, h, :], pss)

            def z_stage_w(h0, zch):
                zw4 = zch[:].rearrange("p (s h q) -> p s h q", h=8, q=WL)
                for h in range(8):
                    ps = s_pool.tile([C, 8 * H], FP32, tag="s")
                    rhs = xp3[:, h0 : h0 + 8, 0:WL]
                    nc.tensor.matmul(ps[:, 0:512], wsb[f"gz{h}_w"][:], rhs)
                    pss = ps[:, 0:512].rearrange("p (s q) -> p s q", q=WL)
                    if h % 2 == 0:
                        nc.scalar.copy(zw4[:, :, h, :], pss)
                    else:
                        nc.vector.tensor_copy(zw4[:, :, h, :], pss)

            def attn_seq(ax, nq, z_seq, xp_key_ap, seq_idx, u2_sl, u2_batch):
                """one attention sequence through attn@v; returns u_ps."""
                # --- scores: S[m, (h, q)] = xp_slice^T @ z_slice
                s_ps = s_pool.tile([C, 8 * H], FP32, tag="s")
                ncols = 8 * nq
                for j in range((ncols + 511) // 512):
                    c0, c1 = j * 512, min((j + 1) * 512, ncols)
                    nc.tensor.matmul(
                        s_ps[:, c0:c1], xp_key_ap, z_seq[:, c0:c1]
                    )
                uv = u_pool.tile([C, 2 * H + C], FP32, tag="u")
                # --- v projection: [keys, 128] (16 dims per head, compact)
                v_ps = uv[:, 2 * H : 2 * H + C]
                nc.tensor.matmul(v_ps, xp_key_ap, wsb[f"wv_{ax}"][:])
                v_sb = v_tiles[seq_idx % 3]
                vsrc = v_ps.rearrange("p (h c) -> p h c", c=16)
                vdst = v_sb[:].rearrange("p (h c) -> p h c", c=32)[:, :, 0:16]
                nc.vector.tensor_copy(vdst, vsrc)

                # --- exp
                et_sb = et_pool.tile([C, 8 * H], BF16, tag="et")
                nc.scalar.activation(et_sb[:, : 8 * nq], s_ps[:, : 8 * nq], EXP)

                # --- attn@v (u) and sums (u2), col-tiled
                u_ps = uv[:, 0 : 2 * H]
                for h in range(8):
                    g, half = h % 4, h // 4
                    oc = slice(half * nq, (half + 1) * nq)
                    nc.tensor.matmul(
                        u_ps[32 * g : 32 * g + 32, oc],
                        v_sb[:, 32 * h : 32 * h + 32],
                        et_sb[:, h * nq : (h + 1) * nq],
                        tile_position=(0, 32 * g),
                    )
                cs = slice(u2_sl * 2 * nq, (u2_sl + 1) * 2 * nq)
                nc.vector.tensor_copy(u2_batch[:, cs], u_ps[:, 0 : 2 * nq])
                return u_ps

            def normalize(u_sb_batch, rb_ps, on_batch, nb, nq):
                w = nb * 2 * nq
                nc.tensor.matmul(rb_ps[:, :w], sel_sb[:], u_sb_batch[:, :w])
                ls_sb = misc_pool.tile([C, NB * 2 * H], FP32, tag="ls")
                rn_sb = misc_pool.tile([C, NB * 2 * H], BF16, tag="rn")
                nc.scalar.activation(ls_sb[:, :w], rb_ps[:, :w], LOG)
                nc.scalar.activation(rn_sb[:, :w], ls_sb[:, :w], EXP, scale=-1.0)
                nc.vector.tensor_tensor(
                    out=on_batch[:, :w],
                    in0=u_sb_batch[:, :w],
                    in1=rn_sb[:, :w],
                    op=MULT,
                )

            def oproj(ax, on_batch, y_ps, nb, nq, y_dst_fn):
                on3 = on_batch[:].rearrange("p (s c) -> p s c", c=2 * nq)
                nc.tensor.matmul(
                    y_ps[:, : nb * nq],
                    wsb[f"woa_{ax}"][:],
                    on3[:, 0:nb, 0:nq],
                    start=True,
                    stop=False,
                )
                nc.tensor.matmul(
                    y_ps[:, : nb * nq],
                    wsb[f"wob_{ax}"][:],
                    on3[:, 0:nb, nq : 2 * nq],
                    start=False,
                    stop=True,
                )
                y_dst_fn(y_ps, nb)

            # ================= H-axis =================
            th3 = th_sb[:].rearrange("p (h w) -> p h w", w=WL)

            for blk in range(WL // NB):
                if blk % (8 // NB) == 0:
                    zch_h = qk_pool.tile([C, 8 * 8 * H], BF16, tag="zh")
                    z_stage_h(blk * NB, zch_h)
                    zh4 = zch_h[:].rearrange("p (s h q) -> p s (h q)", h=8, q=H)
                rby = u2_pool.tile([C, NB * 2 * H + NB * H], FP32, tag="u2")
                rb_ps = rby[:, 0 : NB * 2 * H]
                y_ps = rby[:, NB * 2 * H :]
                usb = on_pool.tile([C, NB * 2 * H], BF16, tag="usb")
                on_batch = on_pool.tile([C, NB * 2 * H], BF16, tag="on")
                for i in range(NB):
                    wl = blk * NB + i
                    attn_seq(
                        "h",
                        nq=H,
                        z_seq=zh4[:, wl % 8, :],
                        xp_key_ap=xp3[:, :, wl],
                        seq_idx=wl,
                        u2_sl=i,
                        u2_batch=usb[:],
                    )
                normalize(usb[:], rb_ps, on_batch[:], NB, nq=H)

                def y_to_th(y_ps, nb, blk=blk):
                    # y cols = (seq wl, q=h); th layout (h, wl)
                    ysrc = y_ps[:, : nb * H].rearrange("p (w h) -> p h w", h=H)
                    nc.vector.tensor_copy(th3[:, :, blk * NB : blk * NB + nb], ysrc)

                oproj("h", on_batch, y_ps, NB, H, y_to_th)

            # ================= W-axis =================
            tw3 = tw_sb[:].rearrange("p (h w) -> p h w", w=WL)

            for blk in range(H // NB):
                if blk % (8 // NB) == 0:
                    zch_w = qk_pool.tile([C, 8 * 8 * WL], BF16, tag="zw")
                    z_stage_w(blk * NB, zch_w)
                    zw4 = zch_w[:].rearrange("p (s h q) -> p s (h q)", h=8, q=WL)
                rby = u2_pool.tile([C, NB * 2 * H + NB * H], FP32, tag="u2")
                rb_ps = rby[:, 0 : NB * 2 * H]
                y_ps = rby[:, NB * 2 * H :]
                usb = on_pool.tile([C, NB * 2 * H], BF16, tag="usb")
                on_batch = on_pool.tile([C, NB * 2 * H], BF16, tag="on")
                for i in range(NB):
                    hr = blk * NB + i
                    attn_seq(
                        "w",
                        nq=WL,
                        z_seq=zw4[:, hr % 8, :],
                        xp_key_ap=xp3[:, hr, :],
                        seq_idx=hr,
                        u2_sl=i,
                        u2_batch=usb[:],
                    )
                normalize(usb[:], rb_ps, on_batch[:], NB, nq=WL)

                def y_to_tw(y_ps, nb, blk=blk):
                    nc.vector.tensor_copy(
                        tw3[:, blk * NB : blk * NB + nb, :],
                        y_ps[:, : nb * WL].rearrange("p (s w) -> p s w", w=WL),
                    )

                oproj("w", on_batch, y_ps, NB, WL, y_to_tw)

            # ================= final: out = th + tw + xres =================
            NFC = 16
            FC = (H * WL) // NFC
            for k in range(NFC):
                cs = slice(k * FC, (k + 1) * FC)
                t1 = misc_pool.tile([C, FC], BF16, tag="f1")
                nc.vector.tensor_tensor(
                    out=t1[:], in0=th_sb[:, cs], in1=tw_sb[:, cs], op=ADD
                )
                o1 = out_pool.tile([C, FC], FP32, tag="o1")
                nc.vector.tensor_tensor(
                    out=o1[:], in0=t1[:], in1=xres_sb[:, cs], op=ADD
                )
                nc.gpsimd.dma_start(out=out[:, cs], in_=o1[:])

    _split_multiwait_drains(nc)
    return nc


_NC_CACHE = None


def _get_nc():
    global _NC_CACHE
    if _NC_CACHE is None:
        _NC_CACHE = _build_nc()
    return _NC_CACHE


def _host_prep(x, pos_h, pos_w, weights, bo_sum):
    scale = DH ** -0.5
    phw = (pos_h + pos_w)[0]  # [C, H, W]

    def grouped_rows(Wm, heads_sel):
        o = np.zeros((C, C), np.float32)
        for g, h in enumerate(heads_sel):
            o[32 * g : 32 * g + 16, :] = Wm[16 * h : 16 * h + 16, :]
        return o

    base = {}
    for ax in AXES:
        Wq, Wk, Wv, Wo = weights[ax]
        for h in range(8):
            G = scale * (Wq[:, 16 * h : 16 * h + 16] @ Wk[:, 16 * h : 16 * h + 16].T)
            base[f"gz{h}_{ax}"] = G.astype(BF16_NP)
        base[f"wv_{ax}"] = Wv.astype(BF16_NP)  # compact: head h at cols 16h..
        woa = grouped_rows(Wo, [0, 1, 2, 3])
        wob = grouped_rows(Wo, [4, 5, 6, 7])
        if ax == "h":
            # bias via the on==1 rows (32g+16): 8 such rows across A+B
            for g in range(4):
                woa[32 * g + 16, :] = bo_sum / 8.0
                wob[32 * g + 16, :] = bo_sum / 8.0
        base[f"woa_{ax}"] = woa.astype(BF16_NP)
        base[f"wob_{ax}"] = wob.astype(BF16_NP)

    selm = np.zeros((C, C), np.float32)
    for q in range(C):
        selm[32 * (q // 32) + 16, q] = 1.0
    base["sel"] = selm.astype(BF16_NP)

    xp_full = (x + phw[None]).astype(BF16_NP)  # [B, C, H, W]

    in_maps = []
    for core in range(N_CORES):
        b, s = core // 2, core % 2
        xb = xp_full[b]
        if s == 1:
            xb = np.concatenate([xb[:, :, WL:], xb[:, :, :WL]], axis=2)
            xr = x[b][:, :, WL:]
        else:
            xr = x[b][:, :, :WL]
        m = dict(base)
        m["xp"] = np.ascontiguousarray(xb.reshape(C, H * W))
        m["xres"] = np.ascontiguousarray(xr.reshape(C, H * WL)).astype(BF16_NP)
        in_maps.append(m)
    return in_maps


LAST_RESULT = None


def kernel(**inputs):
    x = np.asarray(inputs["x"], np.float32)
    pos_h = np.asarray(inputs["pos_h"], np.float32)
    pos_w = np.asarray(inputs["pos_w"], np.float32)
    weights = {
        "h": tuple(np.asarray(inputs[f"W{t}_h"], np.float32) for t in "qkvo"),
        "w": tuple(np.asarray(inputs[f"W{t}_w"], np.float32) for t in "qkvo"),
    }
    bo_sum = np.asarray(inputs["bo_h"], np.float32) + np.asarray(
        inputs["bo_w"], np.float32
    )

    in_maps = _host_prep(x, pos_h, pos_w, weights, bo_sum)

    nc = _get_nc()
    kw = {}
    if os.environ.get("AXIAL_TRACE") == "1":
        kw["trace"] = True
        td = os.environ.get("AXIAL_TMPDIR")
        if td:
            kw["tmpdir"] = td
    res = run_bass_kernel_spmd(nc, in_maps, list(range(N_CORES)), **kw)
    global LAST_RESULT
    LAST_RESULT = res

    out = np.empty((B, C, H, W), np.float32)
    for core in range(N_CORES):
        b, s = core // 2, core % 2
        o = res.results[core]["out"].reshape(C, H, WL)
        out[b, :, :, s * WL : (s + 1) * WL] = o
    return out


if __name__ == "__main__":
    import reference

    inputs = {k: np.asarray(v) for k, v in reference.setup_inputs().items()}
    got = kernel(**inputs)
    import jax

    with jax.default_device(jax.devices("cpu")[0]):
        exp = np.asarray(reference.reference(**reference.setup_inputs()))
    err = np.abs(got - exp).max() / np.abs(exp).max()
    print("rel err:", err)
